# revision 31
# baseline (speedup 1.0000x reference)
"""Multi-head causal attention (QKV proj + RoPE + softmax attention + out proj)
as a distributed Bass kernel on 8 Trainium2 NeuronCores.

Sharding: tensor-parallel over heads. Each core owns 2 of the 16 heads:
it computes Q/K/V for its heads from the full (replicated) input, runs
attention, then the per-head attention outputs (in d-major layout) are
AllGather'd so every core can compute a 256-column slice of the final
output projection. The host concatenates the 8 column slices.

All matmuls run in bf16 (fp32 PSUM accumulation); softmax runs without
max-subtraction (scores are ~N(0,1) here, so exp is safe in fp32).
"""

import math
import numpy as np
import ml_dtypes

B, S, D, H = 2, 2048, 2048, 16
HD = 128                  # head dim
P = 128                   # SBUF partitions
NT = B * S                # 4096 tokens
N_CORES = 8
HPC = H // N_CORES        # heads per core
DQ = HPC * HD             # 256 q/k/v rows per core
KC = D // P               # 16 contraction chunks
TCH = 512                 # token chunk in QKV projection
NTC = NT // TCH           # 8
SBK = S // P              # 16 key blocks per batch
QCH = 512                 # q chunk in attention
NQC = S // QCH            # 4 per batch
SH = S // 2               # AllGather half (1024 tokens)
BF = ml_dtypes.bfloat16

_cache = {}


def _vaug_col(b, i, h):
    # column base of V chunk (batch b, s-chunk i, head h) in the vaug tile
    return ((b * SBK + i) * HPC + h) * (HD + 1)


def _build(mask_mode):
    from concourse import bacc
    import concourse.mybir as mybir
    import concourse.tile as tile
    from concourse.tile_rust import add_dep_helper

    bf = mybir.dt.bfloat16
    f32 = mybir.dt.float32
    EXP = mybir.ActivationFunctionType.Exp
    CPY = mybir.ActivationFunctionType.Copy
    scale = 1.0 / math.sqrt(HD)

    nc = bacc.Bacc("TRN2", target_bir_lowering=False, debug=False,
                   num_devices=N_CORES)

    xT = nc.declare_dram_parameter("xT", [D, NT], bf, isOutput=False)
    wqT = nc.declare_dram_parameter("wqT", [D, DQ], bf, isOutput=False)
    wkT = nc.declare_dram_parameter("wkT", [D, DQ], bf, isOutput=False)
    wvT = nc.declare_dram_parameter("wvT", [D, DQ], bf, isOutput=False)
    woT = nc.declare_dram_parameter("woT", [D, DQ], bf, isOutput=False)
    cro = nc.declare_dram_parameter("cro", [P, NT], bf, isOutput=False)
    sro = nc.declare_dram_parameter("sro", [P, NT], bf, isOutput=False)
    cst = nc.declare_dram_parameter("cst", [P, 3 * P], bf, isOutput=False)
    mskT = None
    if mask_mode == "general":
        mskT = nc.declare_dram_parameter("mskT", [S, S], bf, isOutput=False)
    out = nc.declare_dram_parameter("out", [NT, DQ], f32, isOutput=True)

    rg = [list(range(N_CORES))]

    with tile.TileContext(nc) as tc:
        ptb = 1 if mask_mode == "general" else 2
        with (
            tc.tile_pool(name="per", bufs=1) as per,
            tc.tile_pool(name="stage", bufs=4) as stage,
            tc.tile_pool(name="dram", bufs=1, space="DRAM") as drp,
            tc.tile_pool(name="xs", bufs=20) as xs,
            tc.tile_pool(name="rt", bufs=4) as rt,
            tc.tile_pool(name="ptp", bufs=ptb) as ptp,
            tc.tile_pool(name="ags", bufs=16) as ags,
            tc.tile_pool(name="mkp", bufs=4) as mkp,
            tc.tile_pool(name="psA", bufs=4, space="PSUM") as psA,
            tc.tile_pool(name="ps_st", bufs=2, space="PSUM") as ps_st,
            tc.tile_pool(name="ps_sm", bufs=2, space="PSUM") as ps_sm,
        ):
            # ---------------- persistent SBUF ----------------
            q_sb = per.tile([P, HPC * NT], bf)       # d-major Q, head h at h*NT
            k_sb = per.tile([P, HPC * NT], bf)
            vaug_sb = per.tile([P, B * SBK * HPC * (HD + 1)], bf)
            attn_sb = per.tile([P, HPC * NT], bf)    # d-major attention out
            wo_sb = per.tile([P, KC * DQ], bf)
            wq_sb = per.tile([P, KC * DQ], bf)
            wk_sb = per.tile([P, KC * DQ], bf)
            wv_sb = per.tile([P, KC * DQ], bf)
            cro_sb = per.tile([P, NT], bf)
            sro_sb = per.tile([P, NT], bf)
            cst_sb = per.tile([P, 3 * P], bf)
            ident = cst_sb[:, 0:P]
            perm = cst_sb[:, P:2 * P]
            tri01 = cst_sb[:, 2 * P:3 * P]

            nc.sync.dma_start(out=cst_sb[:], in_=cst[:, :])
            # ones columns for the PV denominator trick
            nc.gpsimd.memset(vaug_sb[:], 1.0)

            # DRAM bounce buffers for the AllGather: one per (batch, s-half)
            bounce = [[drp.tile([DQ, SH], bf, name=f"bounce{b}{f}")
                       for f in range(2)] for b in range(B)]
            ag = [[drp.tile([D, SH], bf, addr_space="Shared", name=f"ag{b}{f}")
                   for f in range(2)] for b in range(B)]

            def chunk(tci):
                """QKV projection + RoPE for one 512-token chunk."""
                t0 = tci * TCH
                xts = []
                for kk in range(KC):
                    if tci == 0:
                        nc.sync.dma_start(
                            out=wv_sb[:, kk * DQ:(kk + 1) * DQ],
                            in_=wvT[kk * P:(kk + 1) * P, :])
                        nc.sync.dma_start(
                            out=wq_sb[:, kk * DQ:(kk + 1) * DQ],
                            in_=wqT[kk * P:(kk + 1) * P, :])
                        nc.sync.dma_start(
                            out=wk_sb[:, kk * DQ:(kk + 1) * DQ],
                            in_=wkT[kk * P:(kk + 1) * P, :])
                    xt = xs.tile([P, TCH], bf, tag="xt", name=f"xt{tci}_{kk}")
                    nc.sync.dma_start(
                        out=xt[:], in_=xT[kk * P:(kk + 1) * P, t0:t0 + TCH])
                    if tci == 0 and kk == 0:
                        nc.sync.dma_start(out=cro_sb[:], in_=cro[:, :])
                        nc.sync.dma_start(out=sro_sb[:], in_=sro[:, :])
                    xts.append(xt)
                # V: two token-blocks share one psum bank; the second group
                # starts with start=False onto the bank cleared by the first
                for u in range(2):
                    vpp = psA.tile([P, 2 * DQ], f32, tag="ps",
                                   name=f"vpp{tci}{u}")
                    first = {}
                    for kk in range(KC):
                        for vh in range(2):
                            tb = u * 2 + vh
                            mm = nc.tensor.matmul(
                                vpp[:, vh * DQ:(vh + 1) * DQ],
                                xts[kk][:, tb * P:(tb + 1) * P],
                                wv_sb[:, kk * DQ:(kk + 1) * DQ],
                                start=(kk == 0 and vh == 0),
                                stop=(kk == KC - 1),
                                skip_group_check=(vh == 1))
                            if kk == 0:
                                first[vh] = mm
                    add_dep_helper(first[1].ins, first[0].ins, sync=False,
                                   reason="bank-clear before second V group")
                    for vh in range(2):
                        tb = u * 2 + vh
                        tglob = t0 + tb * P
                        b = tglob // S
                        i = (tglob % S) // P
                        for h in range(HPC):
                            c0 = _vaug_col(b, i, h)
                            nc.vector.tensor_copy(
                                vaug_sb[:, c0:c0 + HD],
                                vpp[:, vh * DQ + h * HD:vh * DQ + (h + 1) * HD])
                # Q then K sub-loops over the resident xt tiles
                for (w_sb, dst) in ((wq_sb, q_sb), (wk_sb, k_sb)):
                    zp = [psA.tile([P, TCH], f32, tag="ps",
                                   name=f"zp{tci}{dst.name}{m}")
                          for m in range(HPC)]
                    for kk in range(KC):
                        for m in range(HPC):
                            nc.tensor.matmul(
                                zp[m],
                                w_sb[:, kk * DQ + m * HD:kk * DQ + (m + 1) * HD],
                                xts[kk][:], start=(kk == 0), stop=(kk == KC - 1))
                    # RoPE (d-major): out = C*z + S*pairswap(z)
                    for m in range(HPC):
                        zb = stage.tile([P, TCH], bf, tag="zb",
                                        name=f"zb{tci}{m}{dst.name}")
                        nc.scalar.activation(zb[:], zp[m][:], CPY)
                        zs = psA.tile([P, TCH], f32, tag="ps",
                                      name=f"zs{tci}{m}{dst.name}")
                        nc.tensor.matmul(zs[:], perm, zb[:])
                        t1 = rt.tile([P, TCH], f32, tag="t1",
                                     name=f"t1{tci}{m}{dst.name}")
                        t2 = rt.tile([P, TCH], f32, tag="t2",
                                     name=f"t2{tci}{m}{dst.name}")
                        nc.vector.tensor_mul(t1[:], zb[:], cro_sb[:, t0:t0 + TCH])
                        nc.vector.tensor_mul(t2[:], zs[:], sro_sb[:, t0:t0 + TCH])
                        nc.vector.tensor_add(
                            dst[:, m * NT + t0:m * NT + t0 + TCH], t1[:], t2[:])

            def attention(b, half):
                for h in range(HPC):
                    qoff = h * NT + b * S
                    for qc in (half * 2, half * 2 + 1):
                        n_s = SBK if mask_mode != "causal" else 4 * qc + 4
                        pt = ptp.tile([P, SBK * QCH], bf, tag="pt",
                                      name=f"pt{b}{h}{qc}")
                        for sb in range(n_s):
                            stp = ps_st.tile([P, QCH], f32, tag="st",
                                             name=f"st{b}{h}{qc}{sb}")
                            nc.tensor.matmul(
                                stp[:],
                                k_sb[:, qoff + sb * P:qoff + (sb + 1) * P],
                                q_sb[:, qoff + qc * QCH:qoff + (qc + 1) * QCH])
                            if mask_mode == "general":
                                mk = mkp.tile([P, QCH], bf, tag="mk",
                                              name=f"mk{b}{h}{qc}{sb}")
                                nc.sync.dma_start(
                                    out=mk[:],
                                    in_=mskT[sb * P:(sb + 1) * P,
                                             qc * QCH:(qc + 1) * QCH])
                                nc.vector.tensor_add(stp[:], stp[:], mk[:])
                            nc.scalar.activation(
                                pt[:, sb * QCH:(sb + 1) * QCH], stp[:],
                                EXP, scale=scale)
                        if mask_mode == "causal":
                            for j in range(QCH // P):
                                sb = 4 * qc + j
                                c0 = sb * QCH + j * P
                                nc.vector.tensor_mul(
                                    pt[:, c0:c0 + P], pt[:, c0:c0 + P], tri01)
                        for jj in range(QCH // P):
                            qb = 4 * qc + jj
                            n_pv = SBK if mask_mode != "causal" else qb + 1
                            pv = ps_sm.tile([P, HD + 1], f32, tag="sm",
                                            name=f"pv{b}{h}{qb}")
                            for sb in range(n_pv):
                                nc.tensor.matmul(
                                    pv[:],
                                    pt[:, sb * QCH + jj * P:sb * QCH + (jj + 1) * P],
                                    vaug_sb[:, _vaug_col(b, sb, h):
                                            _vaug_col(b, sb, h) + HD + 1],
                                    start=(sb == 0), stop=(sb == n_pv - 1))
                            rec = stage.tile([P, 1], f32, tag="rec",
                                             name=f"rec{b}{h}{qb}")
                            nc.vector.reciprocal(rec[:], pv[:, HD:HD + 1])
                            ast = stage.tile([P, P], bf, tag="ast",
                                             name=f"ast{b}{h}{qb}")
                            nc.vector.tensor_scalar_mul(ast[:], pv[:, 0:HD],
                                                        rec[:])
                            trp = ps_sm.tile([P, P], bf, tag="sm",
                                             name=f"tr{b}{h}{qb}")
                            nc.tensor.transpose(trp[:], ast[:], ident)
                            nc.vector.tensor_copy(
                                attn_sb[:, h * NT + b * S + qb * P:
                                        h * NT + b * S + (qb + 1) * P],
                                trp[:])
                for h in range(HPC):
                    # issued from gpsimd so it doesn't stall the sync stream;
                    # it precedes this half's AllGather there
                    nc.gpsimd.dma_start(
                        out=bounce[b][half][h * HD:(h + 1) * HD, :],
                        in_=attn_sb[:, h * NT + b * S + half * SH:
                                    h * NT + b * S + (half + 1) * SH])
                nc.gpsimd.collective_compute(
                    "AllGather", mybir.AluOpType.bypass,
                    replica_groups=rg,
                    ins=[bounce[b][half].opt()], outs=[ag[b][half].opt()])

            def outproj(b, half):
                for tp in range(SH // (2 * P)):
                    op = [ps_sm.tile([P, DQ], f32, tag="sm",
                                     name=f"op{b}{half}{tp}{u}")
                          for u in range(2)]
                    for kk in range(KC):
                        agt = ags.tile([P, 2 * P], bf, tag="agt",
                                       name=f"agt{b}{half}{tp}{kk}")
                        nc.sync.dma_start(
                            out=agt[:],
                            in_=ag[b][half][kk * P:(kk + 1) * P,
                                            tp * 2 * P:(tp + 1) * 2 * P])
                        for u in range(2):
                            nc.tensor.matmul(
                                op[u], agt[:, u * P:(u + 1) * P],
                                wo_sb[:, kk * DQ:(kk + 1) * DQ],
                                start=(kk == 0), stop=(kk == KC - 1))
                    for u in range(2):
                        ost = stage.tile([P, DQ], f32, tag="ost",
                                         name=f"ost{b}{half}{tp}{u}")
                        nc.vector.tensor_copy(ost[:], op[u][:])
                        trow = b * S + half * SH + (tp * 2 + u) * P
                        nc.gpsimd.dma_start(out=out[trow:trow + P, :],
                                            in_=ost[:])

            for tci in range(4):
                chunk(tci)
            attention(0, 0)
            attention(0, 1)
            for tci in range(4, NTC):
                chunk(tci)
            for kk in range(KC):
                nc.sync.dma_start(out=wo_sb[:, kk * DQ:(kk + 1) * DQ],
                                  in_=woT[kk * P:(kk + 1) * P, :])
            attention(1, 0)
            attention(1, 1)
            outproj(0, 0)
            outproj(0, 1)
            outproj(1, 0)
            outproj(1, 1)

    nc.compile()
    return nc


def _host_prep(inputs):
    x = np.ascontiguousarray(np.asarray(inputs["x"], np.float32).reshape(NT, D))
    wq = np.asarray(inputs["wq"], np.float32)
    wk = np.asarray(inputs["wk"], np.float32)
    wv = np.asarray(inputs["wv"], np.float32)
    wo = np.asarray(inputs["wo"], np.float32)
    cos = np.asarray(inputs["freqs_cos"], np.float32)
    sin = np.asarray(inputs["freqs_sin"], np.float32)
    mask = np.asarray(inputs["mask"], np.float32).reshape(S, S)

    tril = np.tril(np.ones((S, S), bool))
    if not mask.any():
        mode = "zeros"
    elif (mask[tril] == 0).all() and (mask[~tril] <= -1e8).all():
        mode = "causal"
    else:
        mode = "general"

    xT = np.ascontiguousarray(x.T.astype(BF))
    C = np.empty((P, S), np.float32)
    Sn = np.empty((P, S), np.float32)
    C[0::2] = cos.T
    C[1::2] = cos.T
    Sn[0::2] = -sin.T
    Sn[1::2] = sin.T
    cro = np.ascontiguousarray(np.concatenate([C, C], axis=1).astype(BF))
    sro = np.ascontiguousarray(np.concatenate([Sn, Sn], axis=1).astype(BF))

    cst = np.zeros((P, 3 * P), np.float32)
    cst[:, 0:P] = np.eye(P)
    pr = np.zeros((P, P), np.float32)
    idx = np.arange(0, P, 2)
    pr[idx, idx + 1] = 1.0
    pr[idx + 1, idx] = 1.0
    cst[:, P:2 * P] = pr
    cst[:, 2 * P:3 * P] = np.triu(np.ones((P, P), np.float32))
    cst = np.ascontiguousarray(cst.astype(BF))

    in_maps = []
    for c in range(N_CORES):
        r = slice(c * DQ, (c + 1) * DQ)
        m = {
            "xT": xT,
            "wqT": np.ascontiguousarray(wq[r, :].T.astype(BF)),
            "wkT": np.ascontiguousarray(wk[r, :].T.astype(BF)),
            "wvT": np.ascontiguousarray(wv[r, :].T.astype(BF)),
            "woT": np.ascontiguousarray(wo[r, :].T.astype(BF)),
            "cro": cro,
            "sro": sro,
            "cst": cst,
        }
        if mode == "general":
            m["mskT"] = np.ascontiguousarray(
                (mask.T * math.sqrt(HD)).astype(BF))
        in_maps.append(m)
    return mode, in_maps


LAST_RESULT = None


def kernel(**inputs):
    global LAST_RESULT
    from concourse.bass_utils import run_bass_kernel_spmd

    mode, in_maps = _host_prep(inputs)
    if mode not in _cache:
        _cache[mode] = _build(mode)
    nc = _cache[mode]

    res = run_bass_kernel_spmd(nc, in_maps, list(range(N_CORES)))
    LAST_RESULT = res

    out_full = np.empty((NT, D), np.float32)
    for c in range(N_CORES):
        out_full[:, c * DQ:(c + 1) * DQ] = res.results[c]["out"]
    return out_full.reshape(B, S, D)


# revision 32
# speedup vs baseline: 1.0606x; 1.0606x over previous
"""Multi-head causal attention (QKV proj + RoPE + softmax attention + out proj)
as a distributed Bass kernel on 8 Trainium2 NeuronCores.

Sharding: tensor-parallel over heads. Each core owns 2 of the 16 heads:
it computes Q/K/V for its heads from the full (replicated) input, runs
attention, then the per-head attention outputs (in d-major layout) are
AllGather'd so every core can compute a 256-column slice of the final
output projection. The host concatenates the 8 column slices.

All matmuls run in bf16 (fp32 PSUM accumulation); softmax runs without
max-subtraction (scores are ~N(0,1) here, so exp is safe in fp32).
"""

import math
import numpy as np
import ml_dtypes

B, S, D, H = 2, 2048, 2048, 16
HD = 128                  # head dim
P = 128                   # SBUF partitions
NT = B * S                # 4096 tokens
N_CORES = 8
HPC = H // N_CORES        # heads per core
DQ = HPC * HD             # 256 q/k/v rows per core
KC = D // P               # 16 contraction chunks
TCH = 512                 # token chunk in QKV projection
NTC = NT // TCH           # 8
SBK = S // P              # 16 key blocks per batch
QCH = 512                 # q chunk in attention
NQC = S // QCH            # 4 per batch
SH = S // 2               # AllGather half (1024 tokens)
BF = ml_dtypes.bfloat16

_cache = {}


def _vaug_col(b, i, h):
    # column base of V chunk (batch b, s-chunk i, head h) in the vaug tile
    return ((b * SBK + i) * HPC + h) * (HD + 1)


def _build(mask_mode):
    from concourse import bacc
    import concourse.mybir as mybir
    import concourse.tile as tile

    bf = mybir.dt.bfloat16
    f32 = mybir.dt.float32
    EXP = mybir.ActivationFunctionType.Exp
    CPY = mybir.ActivationFunctionType.Copy
    scale = 1.0 / math.sqrt(HD)

    nc = bacc.Bacc("TRN2", target_bir_lowering=False, debug=False,
                   num_devices=N_CORES)

    xT = nc.declare_dram_parameter("xT", [D, NT], bf, isOutput=False)
    wqT = nc.declare_dram_parameter("wqT", [D, DQ], bf, isOutput=False)
    wkT = nc.declare_dram_parameter("wkT", [D, DQ], bf, isOutput=False)
    wvT = nc.declare_dram_parameter("wvT", [D, DQ], bf, isOutput=False)
    woT = nc.declare_dram_parameter("woT", [D, DQ], bf, isOutput=False)
    cro = nc.declare_dram_parameter("cro", [P, NT], bf, isOutput=False)
    sro = nc.declare_dram_parameter("sro", [P, NT], bf, isOutput=False)
    cst = nc.declare_dram_parameter("cst", [P, 3 * P], bf, isOutput=False)
    mskT = None
    if mask_mode == "general":
        mskT = nc.declare_dram_parameter("mskT", [S, S], bf, isOutput=False)
    out = nc.declare_dram_parameter("out", [NT, DQ], f32, isOutput=True)

    rg = [list(range(N_CORES))]

    with tile.TileContext(nc) as tc:
        with (
            tc.tile_pool(name="per", bufs=1) as per,
            tc.tile_pool(name="stage", bufs=4) as stage,
            tc.tile_pool(name="dram", bufs=1, space="DRAM") as drp,
        ):
            # ---------------- persistent SBUF ----------------
            q_sb = per.tile([P, HPC * NT], bf)       # d-major Q, head h at h*NT
            k_sb = per.tile([P, HPC * NT], bf)
            vaug_sb = per.tile([P, B * SBK * HPC * (HD + 1)], bf)
            attn_sb = per.tile([P, HPC * NT], bf)    # d-major attention out
            wo_sb = per.tile([P, KC * DQ], bf)
            cst_sb = per.tile([P, 3 * P], bf)
            ident = cst_sb[:, 0:P]
            perm = cst_sb[:, P:2 * P]
            tri01 = cst_sb[:, 2 * P:3 * P]

            nc.sync.dma_start(out=cst_sb[:], in_=cst[:, :])
            # ones columns for the PV denominator trick
            nc.gpsimd.memset(vaug_sb[:], 1.0)

            # phase-A-scoped SBUF
            wq_sb, free_wq = tc.tile([P, KC * DQ], bf, name="wq_sb")
            wk_sb, free_wk = tc.tile([P, KC * DQ], bf, name="wk_sb")
            wv_sb, free_wv = tc.tile([P, KC * DQ], bf, name="wv_sb")
            cro_sb, free_cro = tc.tile([P, NT], bf, name="cro_sb")
            sro_sb, free_sro = tc.tile([P, NT], bf, name="sro_sb")

            # DRAM bounce buffers for the AllGather: one per (batch, s-half)
            bounce = [[drp.tile([DQ, SH], bf, name=f"bounce{b}{f}")
                       for f in range(2)] for b in range(B)]
            ag = [[drp.tile([D, SH], bf, addr_space="Shared", name=f"ag{b}{f}")
                   for f in range(2)] for b in range(B)]

            # ---------------- phase A: QKV projection + RoPE ----------------
            with (
                tc.tile_pool(name="ps_qk", bufs=4, space="PSUM") as ps_qk,
                tc.tile_pool(name="ps_v", bufs=4, space="PSUM") as ps_v,
                tc.tile_pool(name="xs", bufs=6) as xs,
                tc.tile_pool(name="rt", bufs=4) as rt,
            ):
                for tci in range(NTC):
                    t0 = tci * TCH
                    qp = [ps_qk.tile([P, TCH], f32, tag="qkps", name=f"qp{tci}_{m}")
                          for m in range(HPC)]
                    kp = [ps_qk.tile([P, TCH], f32, tag="qkps", name=f"kp{tci}_{m}")
                          for m in range(HPC)]
                    vp = [ps_v.tile([P, DQ], f32, tag="vps", name=f"vp{tci}_{tb}")
                          for tb in range(TCH // P)]
                    for kk in range(KC):
                        if tci == 0:
                            # stream weights in just ahead of first use
                            nc.sync.dma_start(
                                out=wq_sb[:, kk * DQ:(kk + 1) * DQ],
                                in_=wqT[kk * P:(kk + 1) * P, :])
                            nc.sync.dma_start(
                                out=wk_sb[:, kk * DQ:(kk + 1) * DQ],
                                in_=wkT[kk * P:(kk + 1) * P, :])
                            nc.sync.dma_start(
                                out=wv_sb[:, kk * DQ:(kk + 1) * DQ],
                                in_=wvT[kk * P:(kk + 1) * P, :])
                        xt = xs.tile([P, TCH], bf, tag="xt", name=f"xt{tci}_{kk}")
                        nc.sync.dma_start(
                            out=xt[:], in_=xT[kk * P:(kk + 1) * P, t0:t0 + TCH])
                        st = (kk == 0)
                        sp = (kk == KC - 1)
                        for m in range(HPC):
                            nc.tensor.matmul(
                                qp[m], wq_sb[:, kk * DQ + m * HD:kk * DQ + (m + 1) * HD],
                                xt[:], start=st, stop=sp)
                            nc.tensor.matmul(
                                kp[m], wk_sb[:, kk * DQ + m * HD:kk * DQ + (m + 1) * HD],
                                xt[:], start=st, stop=sp)
                        for tb in range(TCH // P):
                            nc.tensor.matmul(
                                vp[tb], xt[:, tb * P:(tb + 1) * P],
                                wv_sb[:, kk * DQ:(kk + 1) * DQ], start=st, stop=sp)
                        if tci == 0 and kk == 0:
                            # rope tables: needed by first rope, not first MM
                            nc.sync.dma_start(out=cro_sb[:], in_=cro[:, :])
                            nc.sync.dma_start(out=sro_sb[:], in_=sro[:, :])
                    # V: copy token-major psum into vaug (per head), bf16 (DVE)
                    for tb in range(TCH // P):
                        tglob = t0 + tb * P
                        b = tglob // S
                        i = (tglob % S) // P
                        for h in range(HPC):
                            c0 = _vaug_col(b, i, h)
                            nc.vector.tensor_copy(
                                vaug_sb[:, c0:c0 + HD],
                                vp[tb][:, h * HD:(h + 1) * HD])
                    # RoPE on Q and K (d-major): out = C*z + Sro*pairswap(z)
                    for (ps_list, dst) in ((qp, q_sb), (kp, k_sb)):
                        for m in range(HPC):
                            zb = stage.tile([P, TCH], bf, tag="zb", name=f"zb{tci}{m}")
                            nc.scalar.activation(zb[:], ps_list[m][:], CPY)
                            zs = ps_qk.tile([P, TCH], f32, tag="qkps",
                                            name=f"zs{tci}{m}")
                            nc.tensor.matmul(zs[:], perm, zb[:])
                            t1 = rt.tile([P, TCH], f32, tag="t1", name=f"t1{tci}{m}")
                            t2 = rt.tile([P, TCH], f32, tag="t2", name=f"t2{tci}{m}")
                            nc.vector.tensor_mul(t1[:], zb[:], cro_sb[:, t0:t0 + TCH])
                            nc.vector.tensor_mul(t2[:], zs[:], sro_sb[:, t0:t0 + TCH])
                            nc.vector.tensor_add(
                                dst[:, m * NT + t0:m * NT + t0 + TCH], t1[:], t2[:])

            free_sro(); free_cro(); free_wv(); free_wk(); free_wq()

            # out-proj weights: needed only after the first AllGather
            for kk in range(KC):
                nc.sync.dma_start(out=wo_sb[:, kk * DQ:(kk + 1) * DQ],
                                  in_=woT[kk * P:(kk + 1) * P, :])

            # ---------------- phase B+C: attention, AllGather, out-proj ----
            ptb = 1 if mask_mode == "general" else 2
            with (
                tc.tile_pool(name="ps_st", bufs=3, space="PSUM") as ps_st,
                tc.tile_pool(name="ps_pv", bufs=2, space="PSUM") as ps_pv,
                tc.tile_pool(name="ps_tr", bufs=1, space="PSUM") as ps_tr,
                tc.tile_pool(name="ps_op", bufs=2, space="PSUM") as ps_op,
                tc.tile_pool(name="ptp", bufs=ptb) as ptp,
                tc.tile_pool(name="ags", bufs=8) as ags,
                tc.tile_pool(name="mkp", bufs=4) as mkp,
            ):
                def attention(b, half):
                    for h in range(HPC):
                        qoff = h * NT + b * S
                        for qc in (half * 2, half * 2 + 1):
                            n_s = SBK if mask_mode != "causal" else 4 * qc + 4
                            pt = ptp.tile([P, SBK * QCH], bf, tag="pt",
                                          name=f"pt{b}{h}{qc}")
                            for sb in range(n_s):
                                stp = ps_st.tile([P, QCH], f32, tag="st",
                                                 name=f"st{b}{h}{qc}{sb}")
                                nc.tensor.matmul(
                                    stp[:],
                                    k_sb[:, qoff + sb * P:qoff + (sb + 1) * P],
                                    q_sb[:, qoff + qc * QCH:qoff + (qc + 1) * QCH])
                                if mask_mode == "general":
                                    mk = mkp.tile([P, QCH], bf, tag="mk",
                                                  name=f"mk{b}{h}{qc}{sb}")
                                    nc.sync.dma_start(
                                        out=mk[:],
                                        in_=mskT[sb * P:(sb + 1) * P,
                                                 qc * QCH:(qc + 1) * QCH])
                                    nc.vector.tensor_add(stp[:], stp[:], mk[:])
                                nc.scalar.activation(
                                    pt[:, sb * QCH:(sb + 1) * QCH], stp[:],
                                    EXP, scale=scale)
                            if mask_mode == "causal":
                                for j in range(QCH // P):
                                    sb = 4 * qc + j
                                    c0 = sb * QCH + j * P
                                    nc.vector.tensor_mul(
                                        pt[:, c0:c0 + P], pt[:, c0:c0 + P], tri01)
                            for jj in range(QCH // P):
                                qb = 4 * qc + jj
                                n_pv = SBK if mask_mode != "causal" else qb + 1
                                pv = ps_pv.tile([P, HD + 1], f32, tag="pv",
                                                name=f"pv{b}{h}{qb}")
                                for sb in range(n_pv):
                                    nc.tensor.matmul(
                                        pv[:],
                                        pt[:, sb * QCH + jj * P:sb * QCH + (jj + 1) * P],
                                        vaug_sb[:, _vaug_col(b, sb, h):
                                                _vaug_col(b, sb, h) + HD + 1],
                                        start=(sb == 0), stop=(sb == n_pv - 1))
                                rec = stage.tile([P, 1], f32, tag="rec",
                                                 name=f"rec{b}{h}{qb}")
                                nc.vector.reciprocal(rec[:], pv[:, HD:HD + 1])
                                ast = stage.tile([P, P], bf, tag="ast",
                                                 name=f"ast{b}{h}{qb}")
                                nc.vector.tensor_scalar_mul(ast[:], pv[:, 0:HD],
                                                            rec[:])
                                trp = ps_tr.tile([P, P], bf, tag="tr",
                                                 name=f"tr{b}{h}{qb}")
                                nc.tensor.transpose(trp[:], ast[:], ident)
                                nc.vector.tensor_copy(
                                    attn_sb[:, h * NT + b * S + qb * P:
                                            h * NT + b * S + (qb + 1) * P],
                                    trp[:])
                    for h in range(HPC):
                        # issued from gpsimd so it doesn't stall the sync
                        # stream; it precedes this half's AllGather there
                        nc.gpsimd.dma_start(
                            out=bounce[b][half][h * HD:(h + 1) * HD, :],
                            in_=attn_sb[:, h * NT + b * S + half * SH:
                                        h * NT + b * S + (half + 1) * SH])
                    nc.gpsimd.collective_compute(
                        "AllGather", mybir.AluOpType.bypass,
                        replica_groups=rg,
                        ins=[bounce[b][half].opt()], outs=[ag[b][half].opt()])

                def outproj(b, half):
                    for tp in range(SH // (2 * P)):
                        op = [ps_op.tile([P, DQ], f32, tag="op",
                                         name=f"op{b}{half}{tp}{u}")
                              for u in range(2)]
                        for kk in range(KC):
                            agt = ags.tile([P, 2 * P], bf, tag="agt",
                                           name=f"agt{b}{half}{tp}{kk}")
                            nc.sync.dma_start(
                                out=agt[:],
                                in_=ag[b][half][kk * P:(kk + 1) * P,
                                                tp * 2 * P:(tp + 1) * 2 * P])
                            for u in range(2):
                                nc.tensor.matmul(
                                    op[u], agt[:, u * P:(u + 1) * P],
                                    wo_sb[:, kk * DQ:(kk + 1) * DQ],
                                    start=(kk == 0), stop=(kk == KC - 1))
                        for u in range(2):
                            ost = stage.tile([P, DQ], f32, tag="ost",
                                             name=f"ost{b}{half}{tp}{u}")
                            nc.vector.tensor_copy(ost[:], op[u][:])
                            trow = b * S + half * SH + (tp * 2 + u) * P
                            nc.sync.dma_start(out=out[trow:trow + P, :],
                                               in_=ost[:])

                attention(0, 0)
                attention(0, 1)
                attention(1, 0)
                outproj(0, 0)
                attention(1, 1)
                outproj(0, 1)
                outproj(1, 0)
                outproj(1, 1)

    nc.compile()
    return nc


def _host_prep(inputs):
    x = np.ascontiguousarray(np.asarray(inputs["x"], np.float32).reshape(NT, D))
    wq = np.asarray(inputs["wq"], np.float32)
    wk = np.asarray(inputs["wk"], np.float32)
    wv = np.asarray(inputs["wv"], np.float32)
    wo = np.asarray(inputs["wo"], np.float32)
    cos = np.asarray(inputs["freqs_cos"], np.float32)
    sin = np.asarray(inputs["freqs_sin"], np.float32)
    mask = np.asarray(inputs["mask"], np.float32).reshape(S, S)

    tril = np.tril(np.ones((S, S), bool))
    if not mask.any():
        mode = "zeros"
    elif (mask[tril] == 0).all() and (mask[~tril] <= -1e8).all():
        mode = "causal"
    else:
        mode = "general"

    xT = np.ascontiguousarray(x.T.astype(BF))
    C = np.empty((P, S), np.float32)
    Sn = np.empty((P, S), np.float32)
    C[0::2] = cos.T
    C[1::2] = cos.T
    Sn[0::2] = -sin.T
    Sn[1::2] = sin.T
    cro = np.ascontiguousarray(np.concatenate([C, C], axis=1).astype(BF))
    sro = np.ascontiguousarray(np.concatenate([Sn, Sn], axis=1).astype(BF))

    cst = np.zeros((P, 3 * P), np.float32)
    cst[:, 0:P] = np.eye(P)
    pr = np.zeros((P, P), np.float32)
    idx = np.arange(0, P, 2)
    pr[idx, idx + 1] = 1.0
    pr[idx + 1, idx] = 1.0
    cst[:, P:2 * P] = pr
    cst[:, 2 * P:3 * P] = np.triu(np.ones((P, P), np.float32))
    cst = np.ascontiguousarray(cst.astype(BF))

    in_maps = []
    for c in range(N_CORES):
        r = slice(c * DQ, (c + 1) * DQ)
        m = {
            "xT": xT,
            "wqT": np.ascontiguousarray(wq[r, :].T.astype(BF)),
            "wkT": np.ascontiguousarray(wk[r, :].T.astype(BF)),
            "wvT": np.ascontiguousarray(wv[r, :].T.astype(BF)),
            "woT": np.ascontiguousarray(wo[r, :].T.astype(BF)),
            "cro": cro,
            "sro": sro,
            "cst": cst,
        }
        if mode == "general":
            m["mskT"] = np.ascontiguousarray(
                (mask.T * math.sqrt(HD)).astype(BF))
        in_maps.append(m)
    return mode, in_maps


LAST_RESULT = None


def kernel(**inputs):
    global LAST_RESULT
    from concourse.bass_utils import run_bass_kernel_spmd

    mode, in_maps = _host_prep(inputs)
    if mode not in _cache:
        _cache[mode] = _build(mode)
    nc = _cache[mode]

    res = run_bass_kernel_spmd(nc, in_maps, list(range(N_CORES)))
    LAST_RESULT = res

    out_full = np.empty((NT, D), np.float32)
    for c in range(N_CORES):
        out_full[:, c * DQ:(c + 1) * DQ] = res.results[c]["out"]
    return out_full.reshape(B, S, D)
